# revision 14
# baseline (speedup 1.0000x reference)
"""Trainium2 Bass kernel for nn_MixtureCogrammar (v2).

Computation (reference):
    attn  = softmax(morphosyn @ W_affix)                    [B, V]
    affix = attn @ affix_vocab.reshape(V, D*N)              [B, D, N]
    wC    = cumsum_n( sum_{ijk} a_i b_j f_k softmax(pivot_logits[i,j,:,k,:]) )
    out   = stem + wC * (affix - stem)

Distribution: D sharded over 8 cores (DLOC=32); pivot/wC batch-sharded
with an AllGather; attention replicated (cheap).

v2 design (vs the v1 baseline at 228 us):
  - unnormalized-exponential attention: logits are computed twice on the
    PE straight from a DMA-transposed morphosyn tile (no PE transposes):
      b-layout  [b,v]: lhsT=morT chunk, rhs=W    -> exp accum_out = Z_b
      vT-layout [v,b]: lhsT=W chunk,  rhs=morT   -> exp -> fp8 E tiles
    The softmax 1/Z_b is folded into the per-partition scale of the
    ScalarE PSUM drain, so attn is never normalized explicitly.
  - fp8 DoubleRow matmuls (contraction 256/instruction) on E x vocab_fp8,
    weight-stationary inner order (rhs-column loop inside each weight).
  - ScalarE drains PSUM -> bf16 raw (scale=1/Z); DVE does
    sub (raw-stem), then wC-gated mul+add, all bf16 SBUF at 2x.
  - pivot softmax uses exp-with-accum (no DVE reduce), STT chain, scan.
  - collective doorbell as early as possible; the main loop's sub work
    is wC-independent and fills the AllGather latency.
  - vocab resident in SBUF as fp8 (4.2 MB), pivot/morphosyn/W bf16.
"""

import os
import sys

import numpy as np

for _p in ("/opt/trn_rl_repo",):
    if os.path.isdir(_p) and _p not in sys.path:
        sys.path.append(_p)

import concourse.bass as bass  # noqa: E402
import concourse.tile as tile  # noqa: E402
from concourse import bacc, mybir  # noqa: E402
from concourse.bass import ts  # noqa: E402
from concourse.bass_utils import run_bass_kernel_spmd  # noqa: E402

import ml_dtypes  # noqa: E402

B, D, N, DM, V = 1024, 256, 256, 128, 512
NCORES = 8
DLOC = D // NCORES          # 32 d-values per core
BCH = B // 128              # 8 batch chunks
DN = DLOC * N               # 8192 free elems per core
HALF = DN // 2              # 4096 per round (16 d-values)
DHALF = DLOC // 2
PSW = 2048                  # one psum tile = 4 banks

F32 = mybir.dt.float32
BF16 = mybir.dt.bfloat16
FP8 = mybir.dt.float8e4
EXP = mybir.ActivationFunctionType.Exp
COPY = mybir.ActivationFunctionType.Copy
ALU = mybir.AluOpType
DR = mybir.MatmulPerfMode.DoubleRow

LAST_RESULT = None

_CACHE = {}


def _build():
    if "nc" in _CACHE:
        return _CACHE["nc"]

    nc = bacc.Bacc("TRN2", target_bir_lowering=False, debug=False,
                   num_devices=NCORES)

    stem_d = nc.dram_tensor("stem", [B, DLOC, N], BF16, kind="ExternalInput").ap()
    vocab_d = nc.dram_tensor("vocab", [V, DLOC, N], FP8, kind="ExternalInput").ap()
    mor_d = nc.dram_tensor("morpho", [B, DM], BF16, kind="ExternalInput").ap()
    waff_d = nc.dram_tensor("waffix", [DM, V], BF16, kind="ExternalInput").ap()
    pv_d = nc.dram_tensor("pivot", [2, 2, 128, 5, N], BF16, kind="ExternalInput").ap()
    abf_d = nc.dram_tensor("abf", [1, 9], F32, kind="ExternalInput").ap()
    out_d = nc.dram_tensor("out", [B, DLOC, N], BF16, kind="ExternalOutput").ap()

    from contextlib import ExitStack

    with tile.TileContext(nc) as tc, ExitStack() as ctx:
        const = ctx.enter_context(tc.tile_pool(name="const", bufs=1))

        morT = const.tile([128, B], BF16)          # morphosyn^T, resident
        wsb = const.tile([128, V], BF16)           # W_affix, resident
        attnT = const.tile([128, 4, B], FP8)       # E tiles [v_part, vc, b]
        wc_sb = const.tile([128, BCH, N], BF16)    # gathered wC [b_part, cb, n]
        w_bcast = const.tile([128, 20], F32)
        sEb = const.tile([128, BCH], F32)          # Z per (b_part, cb)
        rZ = const.tile([128, BCH], F32)           # 1/Z

        # vocab resident as fp8: [v_part, vc, r, (d n)]
        vqp = ctx.enter_context(tc.tile_pool(name="vq", bufs=1))
        vq = vqp.tile([128, 4, 2, HALF], FP8)

        small = ctx.enter_context(tc.tile_pool(name="small", bufs=1))
        pvp = tc.alloc_tile_pool(name="pv", bufs=1)
        pv = pvp.tile([128, 4, 5, N], BF16)
        pvE = pvp.tile([128, 20, N], F32)

        # ---------- tiny DMAs first: pivot path is the critical path ----
        abf = small.tile([1, 9], F32)
        nc.sync.dma_start(abf[0:1, :], abf_d[:, :])
        for ij in range(4):
            i, j = divmod(ij, 2)
            nc.sync.dma_start(pv[:, ij, :, :], pv_d[i, j, :, :, :])
        nc.sync.dma_start_transpose(morT[:, :], mor_d[:, :])
        nc.sync.dma_start(wsb[:, :], waff_d[:, :])

        # ---------- phase A: mixture weights a (x) b (x) f -> w_bcast ----
        eabf = small.tile([1, 9], F32)
        sums = small.tile([1, 3], F32)
        nc.scalar.activation(eabf[0:1, 0:2], abf[0:1, 0:2], EXP, accum_out=sums[0:1, 0:1])
        nc.scalar.activation(eabf[0:1, 2:4], abf[0:1, 2:4], EXP, accum_out=sums[0:1, 1:2])
        nc.scalar.activation(eabf[0:1, 4:9], abf[0:1, 4:9], EXP, accum_out=sums[0:1, 2:3])
        rsum = small.tile([1, 3], F32)
        nc.vector.reciprocal(rsum[0:1, :], sums[0:1, :])
        t4 = small.tile([1, 4], F32)
        nc.vector.tensor_mul(
            t4[0:1, :].rearrange("p (i j) -> p i j", i=2),
            eabf[0:1, 0:2].rearrange("p (i j) -> p i j", j=1).to_broadcast((1, 2, 2)),
            eabf[0:1, 2:4].rearrange("p (i j) -> p i j", i=1).to_broadcast((1, 2, 2)),
        )
        t20 = small.tile([1, 20], F32)
        nc.vector.tensor_mul(
            t20[0:1, :].rearrange("p (g k) -> p g k", g=4),
            t4[0:1, :].rearrange("p (g k) -> p g k", k=1).to_broadcast((1, 4, 5)),
            eabf[0:1, 4:9].rearrange("p (g k) -> p g k", g=1).to_broadcast((1, 4, 5)),
        )
        rr = small.tile([1, 1], F32)
        nc.vector.tensor_mul(rr[0:1, :], rsum[0:1, 0:1], rsum[0:1, 1:2])
        rrr = small.tile([1, 1], F32)
        nc.vector.tensor_mul(rrr[0:1, :], rr[0:1, :], rsum[0:1, 2:3])
        w20 = small.tile([1, 20], F32)
        nc.vector.tensor_scalar_mul(w20[0:1, :], t20[0:1, :], rrr[0:1, 0:1])
        nc.gpsimd.partition_broadcast(w_bcast[:, :], w20[0:1, :])

        # ---------- phase C: pivot softmaxes -> wC (this core's chunk) ----
        # per-group reciprocal, pipelined against the exps (no global barrier)
        sP = pvp.tile([128, 20], F32)
        rPw = pvp.tile([128, 20], F32)
        accA = pvp.tile([128, N], F32)
        accB = pvp.tile([128, N], F32)
        cur, nxt = accA, accB
        for g in range(20):
            nc.scalar.activation(pvE[:, g, :], pv[:, g // 5, g % 5, :], EXP,
                                 accum_out=sP[:, g:g + 1])
            nc.vector.reciprocal(rPw[:, g:g + 1], sP[:, g:g + 1])
            nc.vector.tensor_mul(rPw[:, g:g + 1], rPw[:, g:g + 1],
                                 w_bcast[:, g:g + 1])
            if g == 0:
                nc.vector.tensor_scalar_mul(accA[:, :], pvE[:, 0, :], rPw[:, 0:1])
            else:
                nc.vector.scalar_tensor_tensor(
                    out=nxt[:, :], in0=pvE[:, g, :], scalar=rPw[:, g:g + 1],
                    in1=cur[:, :], op0=ALU.mult, op1=ALU.add,
                )
                cur, nxt = nxt, cur
        wCl = pvp.tile([128, N], BF16)
        nc.vector.tensor_tensor_scan(
            wCl[:, :], data0=cur[:, :], data1=cur[:, :], initial=0.0,
            op0=ALU.add, op1=ALU.bypass,
        )
        dram = ctx.enter_context(tc.tile_pool(name="dram", bufs=1, space="DRAM"))
        wc_in = dram.tile([128, N], BF16)
        wc_out = nc.dram_tensor("wc_gath", [B, N], BF16, addr_space="Shared").ap()
        nc.sync.dma_start(wc_in[:, :], wCl[:, :])
        nc.gpsimd.collective_compute(
            "AllGather", ALU.bypass,
            replica_groups=[list(range(NCORES))],
            ins=[wc_in[:, :].opt()], outs=[wc_out[:, :].opt()],
        )
        nc.sync.dma_start(
            wc_sb[:, :, :],
            wc_out[:, :].rearrange("(c p) n -> p c n", p=128),
        )

        # ---------- vocab + first stem loads (after the collective's DMA) --
        for vc in range(4):
            for r in range(2):
                nc.sync.dma_start(
                    vq[:, vc, r, :],
                    vocab_d[ts(vc, 128), ts(r, DHALF), :].rearrange("p d n -> p (d n)"),
                )

        # ---------- attention: logits both layouts, exp, Z ----------
        psA = tc.alloc_tile_pool(name="psA", bufs=2, space="PSUM")
        psB = tc.alloc_tile_pool(name="psB", bufs=2, space="PSUM")
        ebp = tc.alloc_tile_pool(name="eb", bufs=2)

        # vT-layout: E tiles for the big matmul (fp8, unnormalized)
        for vc in range(4):
            for bh in range(2):
                lgT = psA.tile([128, 512], F32, tag="lgT", name=f"lgT{vc}_{bh}")
                nc.tensor.matmul(lgT[:, :], lhsT=wsb[:, ts(vc, 128)],
                                 rhs=morT[:, ts(bh, 512)], start=True, stop=True)
                nc.scalar.activation(attnT[:, vc, ts(bh, 512)], lgT[:, :], EXP)
        # b-layout: only for Z_b = sum_v e^logit
        for cb in range(BCH):
            lgb = psB.tile([128, V], F32, tag="lgb", name=f"lgb{cb}")
            nc.tensor.matmul(lgb[:, :], lhsT=morT[:, ts(cb, 128)],
                             rhs=wsb[:, :], start=True, stop=True)
            eb = ebp.tile([128, V], BF16, tag="eb", name=f"eb{cb}")
            nc.scalar.activation(eb[:, :], lgb[:, :], EXP,
                                 accum_out=sEb[:, cb:cb + 1])
        nc.vector.reciprocal(rZ[:, :], sEb[:, :])

        ebp.release()
        pvp.release()
        psB.release()
        psA.release()

        # ---------- main loop ----------
        stp = ctx.enter_context(tc.tile_pool(name="stem", bufs=7))
        rwp = ctx.enter_context(tc.tile_pool(name="raw", bufs=4))
        dlp = ctx.enter_context(tc.tile_pool(name="delta", bufs=13))
        prp = ctx.enter_context(tc.tile_pool(name="prod", bufs=2))
        otp = ctx.enter_context(tc.tile_pool(name="outp", bufs=3))
        psD = ctx.enter_context(tc.tile_pool(name="psD", bufs=2, space="PSUM"))

        tile_idx = 0
        for cb in range(BCH):
            for r in range(2):
                stem_t = stp.tile([128, HALF], BF16)
                nc.sync.dma_start(
                    stem_t[:, :],
                    stem_d[ts(cb, 128), ts(r, DHALF), :].rearrange("p d n -> p (d n)"),
                )
                for h in range(2):
                    ps = psD.tile([128, PSW], F32)
                    # weight-stationary: each DoubleRow pair (contraction 256)
                    # sweeps all 4 psum banks before switching weights
                    for g in range(2):
                        for t in range(PSW // 512):
                            col = h * PSW + t * 512
                            nc.tensor.matmul(
                                ps[:, ts(t, 512)],
                                lhsT=attnT[:, 2 * g:2 * g + 2, ts(cb, 128)],
                                rhs=vq[:, 2 * g:2 * g + 2, r, col:col + 512],
                                start=(g == 0), stop=(g == 1),
                                perf_mode=DR,
                            )
                    raw_t = rwp.tile([128, PSW], BF16)
                    nc.scalar.activation(raw_t[:, :], ps[:, :], COPY,
                                         scale=rZ[:, cb:cb + 1])
                    delta_t = dlp.tile([128, PSW], BF16)
                    sub_eng = nc.gpsimd if tile_idx >= 18 and tile_idx % 2 == 0 \
                        else nc.vector
                    sub_eng.tensor_sub(delta_t[:, :], raw_t[:, :],
                                       stem_t[:, ts(h, PSW)])
                    tile_idx += 1
                    prod = prp.tile([128, PSW], BF16)
                    nc.vector.tensor_mul(
                        prod[:, :].rearrange("p (a n) -> p a n", n=N),
                        delta_t[:, :].rearrange("p (a n) -> p a n", n=N),
                        wc_sb[:, cb:cb + 1, :].to_broadcast((128, PSW // N, N)),
                    )
                    out_t = otp.tile([128, PSW], BF16)
                    nc.vector.tensor_add(out_t[:, :], prod[:, :],
                                         stem_t[:, ts(h, PSW)])
                    nc.sync.dma_start(
                        out_d[ts(cb, 128), bass.ds(r * DHALF + h * (PSW // N), PSW // N), :]
                        .rearrange("p d n -> p (d n)"),
                        out_t[:, :],
                    )

    nc.compile()
    _CACHE["nc"] = nc
    return nc


def kernel(stem_form, morphosyn, pivot_logits, W_affix, affix_vocab,
           alpha, beta, phi, max_len):
    global LAST_RESULT
    stem_form = np.asarray(stem_form, dtype=np.float32)
    morphosyn = np.asarray(morphosyn, dtype=np.float32)
    pivot_logits = np.asarray(pivot_logits, dtype=np.float32)
    W_affix = np.asarray(W_affix, dtype=np.float32)
    affix_vocab = np.asarray(affix_vocab, dtype=np.float32)
    abf = np.concatenate([
        np.asarray(alpha, np.float32).ravel(),
        np.asarray(beta, np.float32).ravel(),
        np.asarray(phi, np.float32).ravel(),
    ]).reshape(1, 9)

    nc = _build()

    stem_np = stem_form.astype(ml_dtypes.bfloat16)
    vocab_np = affix_vocab.astype(ml_dtypes.float8_e4m3)
    mor_np = morphosyn.astype(ml_dtypes.bfloat16)
    waff_np = W_affix.astype(ml_dtypes.bfloat16)
    pv_np = pivot_logits.astype(ml_dtypes.bfloat16)

    in_maps = []
    for c in range(NCORES):
        dlo, dhi = c * DLOC, (c + 1) * DLOC
        in_maps.append({
            "stem": np.ascontiguousarray(stem_np[:, dlo:dhi, :]),
            "vocab": np.ascontiguousarray(vocab_np[:, dlo:dhi, :]),
            "morpho": mor_np,
            "waffix": waff_np,
            "pivot": np.ascontiguousarray(pv_np[:, :, c * 128:(c + 1) * 128, :, :]),
            "abf": abf,
        })

    LAST_RESULT = run_bass_kernel_spmd(nc, in_maps, core_ids=list(range(NCORES)))
    outs = [LAST_RESULT.results[c]["out"] for c in range(NCORES)]
    out = np.concatenate([o.astype(np.float32) for o in outs], axis=1)
    return np.ascontiguousarray(out)


# revision 15
# speedup vs baseline: 1.3340x; 1.3340x over previous
"""Trainium2 Bass kernel for nn_MixtureCogrammar (v2).

Computation (reference):
    attn  = softmax(morphosyn @ W_affix)                    [B, V]
    affix = attn @ affix_vocab.reshape(V, D*N)              [B, D, N]
    wC    = cumsum_n( sum_{ijk} a_i b_j f_k softmax(pivot_logits[i,j,:,k,:]) )
    out   = stem + wC * (affix - stem)

Distribution: D sharded over 8 cores (DLOC=32); pivot/wC batch-sharded
with an AllGather; attention replicated (cheap).

v2 design (vs the v1 baseline at 228 us):
  - unnormalized-exponential attention: logits are computed twice on the
    PE straight from a DMA-transposed morphosyn tile (no PE transposes):
      b-layout  [b,v]: lhsT=morT chunk, rhs=W    -> exp accum_out = Z_b
      vT-layout [v,b]: lhsT=W chunk,  rhs=morT   -> exp -> fp8 E tiles
    The softmax 1/Z_b is folded into the per-partition scale of the
    ScalarE PSUM drain, so attn is never normalized explicitly.
  - fp8 DoubleRow matmuls (contraction 256/instruction) on E x vocab_fp8,
    weight-stationary inner order (rhs-column loop inside each weight).
  - ScalarE drains PSUM -> bf16 raw (scale=1/Z); DVE does
    sub (raw-stem), then wC-gated mul+add, all bf16 SBUF at 2x.
  - pivot softmax uses exp-with-accum (no DVE reduce), STT chain, scan.
  - collective doorbell as early as possible; the main loop's sub work
    is wC-independent and fills the AllGather latency.
  - vocab resident in SBUF as fp8 (4.2 MB), pivot/morphosyn/W bf16.
"""

import os
import sys

import numpy as np

for _p in ("/opt/trn_rl_repo",):
    if os.path.isdir(_p) and _p not in sys.path:
        sys.path.append(_p)

import concourse.bass as bass  # noqa: E402
import concourse.tile as tile  # noqa: E402
from concourse import bacc, mybir  # noqa: E402
from concourse.bass import ts  # noqa: E402
from concourse.bass_utils import run_bass_kernel_spmd  # noqa: E402

import ml_dtypes  # noqa: E402

B, D, N, DM, V = 1024, 256, 256, 128, 512
NCORES = 8
DLOC = D // NCORES          # 32 d-values per core
BCH = B // 128              # 8 batch chunks
DN = DLOC * N               # 8192 free elems per core
HALF = DN // 2              # 4096 per round (16 d-values)
DHALF = DLOC // 2
PSW = 2048                  # one psum tile = 4 banks

F32 = mybir.dt.float32
BF16 = mybir.dt.bfloat16
FP8 = mybir.dt.float8e4
EXP = mybir.ActivationFunctionType.Exp
COPY = mybir.ActivationFunctionType.Copy
ALU = mybir.AluOpType
DR = mybir.MatmulPerfMode.DoubleRow

LAST_RESULT = None

_CACHE = {}


def _build():
    if "nc" in _CACHE:
        return _CACHE["nc"]

    nc = bacc.Bacc("TRN2", target_bir_lowering=False, debug=False,
                   num_devices=NCORES)

    stem_d = nc.dram_tensor("stem", [B, DLOC, N], BF16, kind="ExternalInput").ap()
    vocab_d = nc.dram_tensor("vocab", [V, DLOC, N], FP8, kind="ExternalInput").ap()
    mor_d = nc.dram_tensor("morpho", [B, DM], BF16, kind="ExternalInput").ap()
    waff_d = nc.dram_tensor("waffix", [DM, V], BF16, kind="ExternalInput").ap()
    pv_d = nc.dram_tensor("pivot", [2, 2, 128, 5, N], BF16, kind="ExternalInput").ap()
    abf_d = nc.dram_tensor("abf", [1, 9], F32, kind="ExternalInput").ap()
    out_d = nc.dram_tensor("out", [B, DLOC, N], BF16, kind="ExternalOutput").ap()

    from contextlib import ExitStack

    with tile.TileContext(nc) as tc, ExitStack() as ctx:
        const = ctx.enter_context(tc.tile_pool(name="const", bufs=1))

        morT = const.tile([128, B], BF16)          # morphosyn^T, resident
        wsb = const.tile([128, V], BF16)           # W_affix, resident
        attnT = const.tile([128, 4, B], FP8)       # E tiles [v_part, vc, b]
        wc_sb = const.tile([128, BCH, N], BF16)    # gathered wC [b_part, cb, n]
        w_bcast = const.tile([128, 20], F32)
        sEb = const.tile([128, BCH], F32)          # Z per (b_part, cb)
        rZ = const.tile([128, BCH], F32)           # 1/Z

        # vocab resident as fp8: [v_part, vc, r, (d n)]
        vqp = ctx.enter_context(tc.tile_pool(name="vq", bufs=1))
        vq = vqp.tile([128, 4, 2, HALF], FP8)

        small = ctx.enter_context(tc.tile_pool(name="small", bufs=1))
        pvp = tc.alloc_tile_pool(name="pv", bufs=1)
        pv = pvp.tile([128, 4, 5, N], BF16)
        pvE = pvp.tile([128, 20, N], F32)

        # ---------- tiny DMAs first: pivot path is the critical path ----
        abf = small.tile([1, 9], F32)
        nc.sync.dma_start(abf[0:1, :], abf_d[:, :])
        for ij in range(4):
            i, j = divmod(ij, 2)
            nc.sync.dma_start(pv[:, ij, :, :], pv_d[i, j, :, :, :])
        nc.sync.dma_start_transpose(morT[:, :], mor_d[:, :])
        nc.sync.dma_start(wsb[:, :], waff_d[:, :])

        # ---------- phase A: mixture weights a (x) b (x) f -> w_bcast ----
        eabf = small.tile([1, 9], F32)
        sums = small.tile([1, 3], F32)
        nc.scalar.activation(eabf[0:1, 0:2], abf[0:1, 0:2], EXP, accum_out=sums[0:1, 0:1])
        nc.scalar.activation(eabf[0:1, 2:4], abf[0:1, 2:4], EXP, accum_out=sums[0:1, 1:2])
        nc.scalar.activation(eabf[0:1, 4:9], abf[0:1, 4:9], EXP, accum_out=sums[0:1, 2:3])
        rsum = small.tile([1, 3], F32)
        nc.vector.reciprocal(rsum[0:1, :], sums[0:1, :])
        t4 = small.tile([1, 4], F32)
        nc.vector.tensor_mul(
            t4[0:1, :].rearrange("p (i j) -> p i j", i=2),
            eabf[0:1, 0:2].rearrange("p (i j) -> p i j", j=1).to_broadcast((1, 2, 2)),
            eabf[0:1, 2:4].rearrange("p (i j) -> p i j", i=1).to_broadcast((1, 2, 2)),
        )
        t20 = small.tile([1, 20], F32)
        nc.vector.tensor_mul(
            t20[0:1, :].rearrange("p (g k) -> p g k", g=4),
            t4[0:1, :].rearrange("p (g k) -> p g k", k=1).to_broadcast((1, 4, 5)),
            eabf[0:1, 4:9].rearrange("p (g k) -> p g k", g=1).to_broadcast((1, 4, 5)),
        )
        rr = small.tile([1, 1], F32)
        nc.vector.tensor_mul(rr[0:1, :], rsum[0:1, 0:1], rsum[0:1, 1:2])
        rrr = small.tile([1, 1], F32)
        nc.vector.tensor_mul(rrr[0:1, :], rr[0:1, :], rsum[0:1, 2:3])
        w20 = small.tile([1, 20], F32)
        nc.vector.tensor_scalar_mul(w20[0:1, :], t20[0:1, :], rrr[0:1, 0:1])
        nc.gpsimd.partition_broadcast(w_bcast[:, :], w20[0:1, :])

        # ---------- phase C: pivot softmaxes -> wC (this core's chunk) ----
        sP = pvp.tile([128, 20], F32)
        for g in range(20):
            nc.scalar.activation(pvE[:, g, :], pv[:, g // 5, g % 5, :], EXP,
                                 accum_out=sP[:, g:g + 1])
        rP = pvp.tile([128, 20], F32)
        nc.vector.reciprocal(rP[:, :], sP[:, :])
        rPw = pvp.tile([128, 20], F32)
        nc.vector.tensor_mul(rPw[:, :], rP[:, :], w_bcast[:, :])
        accA = pvp.tile([128, N], F32)
        accB = pvp.tile([128, N], F32)
        nc.vector.tensor_scalar_mul(accA[:, :], pvE[:, 0, :], rPw[:, 0:1])
        cur, nxt = accA, accB
        for g in range(1, 20):
            nc.vector.scalar_tensor_tensor(
                out=nxt[:, :], in0=pvE[:, g, :], scalar=rPw[:, g:g + 1],
                in1=cur[:, :], op0=ALU.mult, op1=ALU.add,
            )
            cur, nxt = nxt, cur
        wCl = pvp.tile([128, N], BF16)
        nc.vector.tensor_tensor_scan(
            wCl[:, :], data0=cur[:, :], data1=cur[:, :], initial=0.0,
            op0=ALU.add, op1=ALU.bypass,
        )
        dram = ctx.enter_context(tc.tile_pool(name="dram", bufs=1, space="DRAM"))
        wc_in = dram.tile([128, N], BF16)
        wc_out = nc.dram_tensor("wc_gath", [B, N], BF16, addr_space="Shared").ap()
        nc.sync.dma_start(wc_in[:, :], wCl[:, :])
        nc.gpsimd.collective_compute(
            "AllGather", ALU.bypass,
            replica_groups=[list(range(NCORES))],
            ins=[wc_in[:, :].opt()], outs=[wc_out[:, :].opt()],
        )
        nc.sync.dma_start(
            wc_sb[:, :, :],
            wc_out[:, :].rearrange("(c p) n -> p c n", p=128),
        )

        # ---------- vocab + first stem loads (after the collective's DMA) --
        for vc in range(4):
            for r in range(2):
                nc.sync.dma_start(
                    vq[:, vc, r, :],
                    vocab_d[ts(vc, 128), ts(r, DHALF), :].rearrange("p d n -> p (d n)"),
                )

        # ---------- attention: logits both layouts, exp, Z ----------
        psA = tc.alloc_tile_pool(name="psA", bufs=2, space="PSUM")
        psB = tc.alloc_tile_pool(name="psB", bufs=2, space="PSUM")
        ebp = tc.alloc_tile_pool(name="eb", bufs=2)

        # vT-layout: E tiles for the big matmul (fp8, unnormalized)
        for vc in range(4):
            for bh in range(2):
                lgT = psA.tile([128, 512], F32, tag="lgT", name=f"lgT{vc}_{bh}")
                nc.tensor.matmul(lgT[:, :], lhsT=wsb[:, ts(vc, 128)],
                                 rhs=morT[:, ts(bh, 512)], start=True, stop=True)
                nc.scalar.activation(attnT[:, vc, ts(bh, 512)], lgT[:, :], EXP)
        # b-layout: only for Z_b = sum_v e^logit
        for cb in range(BCH):
            lgb = psB.tile([128, V], F32, tag="lgb", name=f"lgb{cb}")
            nc.tensor.matmul(lgb[:, :], lhsT=morT[:, ts(cb, 128)],
                             rhs=wsb[:, :], start=True, stop=True)
            eb = ebp.tile([128, V], BF16, tag="eb", name=f"eb{cb}")
            nc.scalar.activation(eb[:, :], lgb[:, :], EXP,
                                 accum_out=sEb[:, cb:cb + 1])
        nc.vector.reciprocal(rZ[:, :], sEb[:, :])

        ebp.release()
        pvp.release()
        psB.release()
        psA.release()

        # ---------- main loop ----------
        stp = ctx.enter_context(tc.tile_pool(name="stem", bufs=6))
        rwp = ctx.enter_context(tc.tile_pool(name="raw", bufs=3))
        dlp = ctx.enter_context(tc.tile_pool(name="delta", bufs=10))
        prp = ctx.enter_context(tc.tile_pool(name="prod", bufs=2))
        otp = ctx.enter_context(tc.tile_pool(name="outp", bufs=3))
        psD = ctx.enter_context(tc.tile_pool(name="psD", bufs=2, space="PSUM"))

        for cb in range(BCH):
            for r in range(2):
                stem_t = stp.tile([128, HALF], BF16)
                nc.sync.dma_start(
                    stem_t[:, :],
                    stem_d[ts(cb, 128), ts(r, DHALF), :].rearrange("p d n -> p (d n)"),
                )
                for h in range(2):
                    ps = psD.tile([128, PSW], F32)
                    # weight-stationary: each DoubleRow pair (contraction 256)
                    # sweeps all 4 psum banks before switching weights
                    for g in range(2):
                        for t in range(PSW // 512):
                            col = h * PSW + t * 512
                            nc.tensor.matmul(
                                ps[:, ts(t, 512)],
                                lhsT=attnT[:, 2 * g:2 * g + 2, ts(cb, 128)],
                                rhs=vq[:, 2 * g:2 * g + 2, r, col:col + 512],
                                start=(g == 0), stop=(g == 1),
                                perf_mode=DR,
                            )
                    raw_t = rwp.tile([128, PSW], BF16)
                    nc.scalar.activation(raw_t[:, :], ps[:, :], COPY,
                                         scale=rZ[:, cb:cb + 1])
                    delta_t = dlp.tile([128, PSW], BF16)
                    nc.vector.tensor_sub(delta_t[:, :], raw_t[:, :],
                                         stem_t[:, ts(h, PSW)])
                    prod = prp.tile([128, PSW], BF16)
                    nc.vector.tensor_mul(
                        prod[:, :].rearrange("p (a n) -> p a n", n=N),
                        delta_t[:, :].rearrange("p (a n) -> p a n", n=N),
                        wc_sb[:, cb:cb + 1, :].to_broadcast((128, PSW // N, N)),
                    )
                    out_t = otp.tile([128, PSW], BF16)
                    nc.vector.tensor_add(out_t[:, :], prod[:, :],
                                         stem_t[:, ts(h, PSW)])
                    nc.sync.dma_start(
                        out_d[ts(cb, 128), bass.ds(r * DHALF + h * (PSW // N), PSW // N), :]
                        .rearrange("p d n -> p (d n)"),
                        out_t[:, :],
                    )

    nc.compile()
    _CACHE["nc"] = nc
    return nc


def kernel(stem_form, morphosyn, pivot_logits, W_affix, affix_vocab,
           alpha, beta, phi, max_len):
    global LAST_RESULT
    stem_form = np.asarray(stem_form, dtype=np.float32)
    morphosyn = np.asarray(morphosyn, dtype=np.float32)
    pivot_logits = np.asarray(pivot_logits, dtype=np.float32)
    W_affix = np.asarray(W_affix, dtype=np.float32)
    affix_vocab = np.asarray(affix_vocab, dtype=np.float32)
    abf = np.concatenate([
        np.asarray(alpha, np.float32).ravel(),
        np.asarray(beta, np.float32).ravel(),
        np.asarray(phi, np.float32).ravel(),
    ]).reshape(1, 9)

    nc = _build()

    stem_np = stem_form.astype(ml_dtypes.bfloat16)
    vocab_np = affix_vocab.astype(ml_dtypes.float8_e4m3)
    mor_np = morphosyn.astype(ml_dtypes.bfloat16)
    waff_np = W_affix.astype(ml_dtypes.bfloat16)
    pv_np = pivot_logits.astype(ml_dtypes.bfloat16)

    in_maps = []
    for c in range(NCORES):
        dlo, dhi = c * DLOC, (c + 1) * DLOC
        in_maps.append({
            "stem": np.ascontiguousarray(stem_np[:, dlo:dhi, :]),
            "vocab": np.ascontiguousarray(vocab_np[:, dlo:dhi, :]),
            "morpho": mor_np,
            "waffix": waff_np,
            "pivot": np.ascontiguousarray(pv_np[:, :, c * 128:(c + 1) * 128, :, :]),
            "abf": abf,
        })

    LAST_RESULT = run_bass_kernel_spmd(nc, in_maps, core_ids=list(range(NCORES)))
    outs = [LAST_RESULT.results[c]["out"] for c in range(NCORES)]
    out = np.concatenate([o.astype(np.float32) for o in outs], axis=1)
    return np.ascontiguousarray(out)


# revision 16
# speedup vs baseline: 1.4065x; 1.0543x over previous
"""Trainium2 Bass kernel for nn_MixtureCogrammar (v2).

Computation (reference):
    attn  = softmax(morphosyn @ W_affix)                    [B, V]
    affix = attn @ affix_vocab.reshape(V, D*N)              [B, D, N]
    wC    = cumsum_n( sum_{ijk} a_i b_j f_k softmax(pivot_logits[i,j,:,k,:]) )
    out   = stem + wC * (affix - stem)

Distribution: D sharded over 8 cores (DLOC=32); pivot/wC batch-sharded
with an AllGather; attention replicated (cheap).

v2 design (vs the v1 baseline at 228 us):
  - unnormalized-exponential attention: logits are computed twice on the
    PE straight from a DMA-transposed morphosyn tile (no PE transposes):
      b-layout  [b,v]: lhsT=morT chunk, rhs=W    -> exp accum_out = Z_b
      vT-layout [v,b]: lhsT=W chunk,  rhs=morT   -> exp -> fp8 E tiles
    The softmax 1/Z_b is folded into the per-partition scale of the
    ScalarE PSUM drain, so attn is never normalized explicitly.
  - fp8 DoubleRow matmuls (contraction 256/instruction) on E x vocab_fp8,
    weight-stationary inner order (rhs-column loop inside each weight).
  - ScalarE drains PSUM -> bf16 raw (scale=1/Z); DVE does
    sub (raw-stem), then wC-gated mul+add, all bf16 SBUF at 2x.
  - pivot softmax uses exp-with-accum (no DVE reduce), STT chain, scan.
  - collective doorbell as early as possible; the main loop's sub work
    is wC-independent and fills the AllGather latency.
  - vocab resident in SBUF as fp8 (4.2 MB), pivot/morphosyn/W bf16.
"""

import os
import sys

import numpy as np

for _p in ("/opt/trn_rl_repo",):
    if os.path.isdir(_p) and _p not in sys.path:
        sys.path.append(_p)

import concourse.bass as bass  # noqa: E402
import concourse.tile as tile  # noqa: E402
from concourse import bacc, mybir  # noqa: E402
from concourse.bass import ts  # noqa: E402
from concourse.bass_utils import run_bass_kernel_spmd  # noqa: E402

import ml_dtypes  # noqa: E402

B, D, N, DM, V = 1024, 256, 256, 128, 512
NCORES = 8
DLOC = D // NCORES          # 32 d-values per core
BCH = B // 128              # 8 batch chunks
DN = DLOC * N               # 8192 free elems per core
HALF = DN // 2              # 4096 per round (16 d-values)
DHALF = DLOC // 2
PSW = 2048                  # one psum tile = 4 banks

F32 = mybir.dt.float32
BF16 = mybir.dt.bfloat16
FP8 = mybir.dt.float8e4
EXP = mybir.ActivationFunctionType.Exp
COPY = mybir.ActivationFunctionType.Copy
ALU = mybir.AluOpType
DR = mybir.MatmulPerfMode.DoubleRow

LAST_RESULT = None

_CACHE = {}


def _build():
    if "nc" in _CACHE:
        return _CACHE["nc"]

    nc = bacc.Bacc("TRN2", target_bir_lowering=False, debug=False,
                   num_devices=NCORES)

    stem_d = nc.dram_tensor("stem", [B, DLOC, N], BF16, kind="ExternalInput").ap()
    vocab_d = nc.dram_tensor("vocab", [V, DLOC, N], FP8, kind="ExternalInput").ap()
    mor_d = nc.dram_tensor("morpho", [B, DM], BF16, kind="ExternalInput").ap()
    waff_d = nc.dram_tensor("waffix", [DM, V], BF16, kind="ExternalInput").ap()
    pv_d = nc.dram_tensor("pivot", [2, 2, 128, 5, N], BF16, kind="ExternalInput").ap()
    abf_d = nc.dram_tensor("abf", [1, 9], F32, kind="ExternalInput").ap()
    out_d = nc.dram_tensor("out", [B, DLOC, N], BF16, kind="ExternalOutput").ap()

    from contextlib import ExitStack

    with tile.TileContext(nc) as tc, ExitStack() as ctx:
        const = ctx.enter_context(tc.tile_pool(name="const", bufs=1))

        morT = const.tile([128, B], BF16)          # morphosyn^T, resident
        wsb = const.tile([128, V], BF16)           # W_affix, resident
        attnT = const.tile([128, 4, B], FP8)       # E tiles [v_part, vc, b]
        wc_sb = const.tile([128, BCH, N], BF16)    # gathered wC [b_part, cb, n]
        w_bcast = const.tile([128, 20], F32)
        sEb = const.tile([128, BCH], F32)          # Z per (b_part, cb)
        rZ = const.tile([128, BCH], F32)           # 1/Z

        # vocab resident as fp8: [v_part, vc, r, (d n)]
        vqp = ctx.enter_context(tc.tile_pool(name="vq", bufs=1))
        vq = vqp.tile([128, 4, 2, HALF], FP8)

        small = ctx.enter_context(tc.tile_pool(name="small", bufs=1))
        pvp = tc.alloc_tile_pool(name="pv", bufs=1)
        pv = pvp.tile([128, 4, 5, N], BF16)
        pvE = pvp.tile([128, 20, N], F32)

        # ---------- tiny DMAs first: pivot path is the critical path ----
        abf = small.tile([1, 9], F32)
        nc.sync.dma_start(abf[0:1, :], abf_d[:, :])
        for ij in range(4):
            i, j = divmod(ij, 2)
            nc.sync.dma_start(pv[:, ij, :, :], pv_d[i, j, :, :, :])
        nc.sync.dma_start_transpose(morT[:, :], mor_d[:, :])
        nc.sync.dma_start(wsb[:, :], waff_d[:, :])

        # ---------- phase A: mixture weights a (x) b (x) f -> w_bcast ----
        eabf = small.tile([1, 9], F32)
        sums = small.tile([1, 3], F32)
        nc.scalar.activation(eabf[0:1, 0:2], abf[0:1, 0:2], EXP, accum_out=sums[0:1, 0:1])
        nc.scalar.activation(eabf[0:1, 2:4], abf[0:1, 2:4], EXP, accum_out=sums[0:1, 1:2])
        nc.scalar.activation(eabf[0:1, 4:9], abf[0:1, 4:9], EXP, accum_out=sums[0:1, 2:3])
        rsum = small.tile([1, 3], F32)
        nc.vector.reciprocal(rsum[0:1, :], sums[0:1, :])
        t4 = small.tile([1, 4], F32)
        nc.vector.tensor_mul(
            t4[0:1, :].rearrange("p (i j) -> p i j", i=2),
            eabf[0:1, 0:2].rearrange("p (i j) -> p i j", j=1).to_broadcast((1, 2, 2)),
            eabf[0:1, 2:4].rearrange("p (i j) -> p i j", i=1).to_broadcast((1, 2, 2)),
        )
        t20 = small.tile([1, 20], F32)
        nc.vector.tensor_mul(
            t20[0:1, :].rearrange("p (g k) -> p g k", g=4),
            t4[0:1, :].rearrange("p (g k) -> p g k", k=1).to_broadcast((1, 4, 5)),
            eabf[0:1, 4:9].rearrange("p (g k) -> p g k", g=1).to_broadcast((1, 4, 5)),
        )
        rr = small.tile([1, 1], F32)
        nc.vector.tensor_mul(rr[0:1, :], rsum[0:1, 0:1], rsum[0:1, 1:2])
        rrr = small.tile([1, 1], F32)
        nc.vector.tensor_mul(rrr[0:1, :], rr[0:1, :], rsum[0:1, 2:3])
        w20 = small.tile([1, 20], F32)
        nc.vector.tensor_scalar_mul(w20[0:1, :], t20[0:1, :], rrr[0:1, 0:1])
        nc.gpsimd.partition_broadcast(w_bcast[:, :], w20[0:1, :])

        # ---------- phase C: pivot softmaxes -> wC (this core's chunk) ----
        # per-group reciprocal, pipelined against the exps (no global barrier)
        sP = pvp.tile([128, 20], F32)
        rPw = pvp.tile([128, 20], F32)
        accA = pvp.tile([128, N], F32)
        accB = pvp.tile([128, N], F32)
        cur, nxt = accA, accB
        for g in range(20):
            nc.scalar.activation(pvE[:, g, :], pv[:, g // 5, g % 5, :], EXP,
                                 accum_out=sP[:, g:g + 1])
            nc.vector.reciprocal(rPw[:, g:g + 1], sP[:, g:g + 1])
            nc.vector.tensor_mul(rPw[:, g:g + 1], rPw[:, g:g + 1],
                                 w_bcast[:, g:g + 1])
            if g == 0:
                nc.vector.tensor_scalar_mul(accA[:, :], pvE[:, 0, :], rPw[:, 0:1])
            else:
                nc.vector.scalar_tensor_tensor(
                    out=nxt[:, :], in0=pvE[:, g, :], scalar=rPw[:, g:g + 1],
                    in1=cur[:, :], op0=ALU.mult, op1=ALU.add,
                )
                cur, nxt = nxt, cur
        wCl = pvp.tile([128, N], BF16)
        nc.vector.tensor_tensor_scan(
            wCl[:, :], data0=cur[:, :], data1=cur[:, :], initial=0.0,
            op0=ALU.add, op1=ALU.bypass,
        )
        dram = ctx.enter_context(tc.tile_pool(name="dram", bufs=1, space="DRAM"))
        wc_in = dram.tile([128, N], BF16)
        wc_out = nc.dram_tensor("wc_gath", [B, N], BF16, addr_space="Shared").ap()
        nc.sync.dma_start(wc_in[:, :], wCl[:, :])
        nc.gpsimd.collective_compute(
            "AllGather", ALU.bypass,
            replica_groups=[list(range(NCORES))],
            ins=[wc_in[:, :].opt()], outs=[wc_out[:, :].opt()],
        )
        nc.sync.dma_start(
            wc_sb[:, :, :],
            wc_out[:, :].rearrange("(c p) n -> p c n", p=128),
        )

        # ---------- vocab + first stem loads (after the collective's DMA) --
        for vc in range(4):
            for r in range(2):
                nc.sync.dma_start(
                    vq[:, vc, r, :],
                    vocab_d[ts(vc, 128), ts(r, DHALF), :].rearrange("p d n -> p (d n)"),
                )

        # ---------- attention: logits both layouts, exp, Z ----------
        psA = tc.alloc_tile_pool(name="psA", bufs=2, space="PSUM")
        psB = tc.alloc_tile_pool(name="psB", bufs=2, space="PSUM")
        ebp = tc.alloc_tile_pool(name="eb", bufs=2)

        # vT-layout: E tiles for the big matmul (fp8, unnormalized)
        for vc in range(4):
            for bh in range(2):
                lgT = psA.tile([128, 512], F32, tag="lgT", name=f"lgT{vc}_{bh}")
                nc.tensor.matmul(lgT[:, :], lhsT=wsb[:, ts(vc, 128)],
                                 rhs=morT[:, ts(bh, 512)], start=True, stop=True)
                nc.scalar.activation(attnT[:, vc, ts(bh, 512)], lgT[:, :], EXP)
        # b-layout: only for Z_b = sum_v e^logit
        for cb in range(BCH):
            lgb = psB.tile([128, V], F32, tag="lgb", name=f"lgb{cb}")
            nc.tensor.matmul(lgb[:, :], lhsT=morT[:, ts(cb, 128)],
                             rhs=wsb[:, :], start=True, stop=True)
            eb = ebp.tile([128, V], BF16, tag="eb", name=f"eb{cb}")
            nc.scalar.activation(eb[:, :], lgb[:, :], EXP,
                                 accum_out=sEb[:, cb:cb + 1])
        nc.vector.reciprocal(rZ[:, :], sEb[:, :])

        ebp.release()
        pvp.release()
        psB.release()
        psA.release()

        # ---------- main loop ----------
        stp = ctx.enter_context(tc.tile_pool(name="stem", bufs=6))
        rwp = ctx.enter_context(tc.tile_pool(name="raw", bufs=2))
        dlp = ctx.enter_context(tc.tile_pool(name="delta", bufs=6))
        prp = ctx.enter_context(tc.tile_pool(name="prod", bufs=2))
        otp = ctx.enter_context(tc.tile_pool(name="outp", bufs=2))
        psD = ctx.enter_context(tc.tile_pool(name="psD", bufs=2, space="PSUM"))

        for cb in range(BCH):
            for r in range(2):
                stem_t = stp.tile([128, HALF], BF16)
                nc.sync.dma_start(
                    stem_t[:, :],
                    stem_d[ts(cb, 128), ts(r, DHALF), :].rearrange("p d n -> p (d n)"),
                )
                raw_t = rwp.tile([128, HALF], BF16)
                for h in range(2):
                    ps = psD.tile([128, PSW], F32)
                    # weight-stationary: each DoubleRow pair (contraction 256)
                    # sweeps all 4 psum banks before switching weights
                    for g in range(2):
                        for t in range(PSW // 512):
                            col = h * PSW + t * 512
                            nc.tensor.matmul(
                                ps[:, ts(t, 512)],
                                lhsT=attnT[:, 2 * g:2 * g + 2, ts(cb, 128)],
                                rhs=vq[:, 2 * g:2 * g + 2, r, col:col + 512],
                                start=(g == 0), stop=(g == 1),
                                perf_mode=DR,
                            )
                    nc.scalar.activation(raw_t[:, ts(h, PSW)], ps[:, :], COPY,
                                         scale=rZ[:, cb:cb + 1])
                # full-round [128,4096] elementwise: half the DVE op count
                delta_t = dlp.tile([128, HALF], BF16)
                nc.vector.tensor_sub(delta_t[:, :], raw_t[:, :], stem_t[:, :])
                prod = prp.tile([128, HALF], BF16)
                nc.vector.tensor_mul(
                    prod[:, :].rearrange("p (a n) -> p a n", n=N),
                    delta_t[:, :].rearrange("p (a n) -> p a n", n=N),
                    wc_sb[:, cb:cb + 1, :].to_broadcast((128, HALF // N, N)),
                )
                out_t = otp.tile([128, HALF], BF16)
                nc.vector.tensor_add(out_t[:, :], prod[:, :], stem_t[:, :])
                nc.sync.dma_start(
                    out_d[ts(cb, 128), bass.ds(r * DHALF, DHALF), :]
                    .rearrange("p d n -> p (d n)"),
                    out_t[:, :],
                )

    nc.compile()
    _CACHE["nc"] = nc
    return nc


def kernel(stem_form, morphosyn, pivot_logits, W_affix, affix_vocab,
           alpha, beta, phi, max_len):
    global LAST_RESULT
    stem_form = np.asarray(stem_form, dtype=np.float32)
    morphosyn = np.asarray(morphosyn, dtype=np.float32)
    pivot_logits = np.asarray(pivot_logits, dtype=np.float32)
    W_affix = np.asarray(W_affix, dtype=np.float32)
    affix_vocab = np.asarray(affix_vocab, dtype=np.float32)
    abf = np.concatenate([
        np.asarray(alpha, np.float32).ravel(),
        np.asarray(beta, np.float32).ravel(),
        np.asarray(phi, np.float32).ravel(),
    ]).reshape(1, 9)

    nc = _build()

    stem_np = stem_form.astype(ml_dtypes.bfloat16)
    vocab_np = affix_vocab.astype(ml_dtypes.float8_e4m3)
    mor_np = morphosyn.astype(ml_dtypes.bfloat16)
    waff_np = W_affix.astype(ml_dtypes.bfloat16)
    pv_np = pivot_logits.astype(ml_dtypes.bfloat16)

    in_maps = []
    for c in range(NCORES):
        dlo, dhi = c * DLOC, (c + 1) * DLOC
        in_maps.append({
            "stem": np.ascontiguousarray(stem_np[:, dlo:dhi, :]),
            "vocab": np.ascontiguousarray(vocab_np[:, dlo:dhi, :]),
            "morpho": mor_np,
            "waffix": waff_np,
            "pivot": np.ascontiguousarray(pv_np[:, :, c * 128:(c + 1) * 128, :, :]),
            "abf": abf,
        })

    LAST_RESULT = run_bass_kernel_spmd(nc, in_maps, core_ids=list(range(NCORES)))
    outs = [LAST_RESULT.results[c]["out"] for c in range(NCORES)]
    out = np.concatenate([o.astype(np.float32) for o in outs], axis=1)
    return np.ascontiguousarray(out)


# revision 17
# speedup vs baseline: 1.5814x; 1.1244x over previous
"""Trainium2 Bass kernel for nn_MixtureCogrammar (v2).

Computation (reference):
    attn  = softmax(morphosyn @ W_affix)                    [B, V]
    affix = attn @ affix_vocab.reshape(V, D*N)              [B, D, N]
    wC    = cumsum_n( sum_{ijk} a_i b_j f_k softmax(pivot_logits[i,j,:,k,:]) )
    out   = stem + wC * (affix - stem)

Distribution: D sharded over 8 cores (DLOC=32); pivot/wC batch-sharded
with an AllGather; attention replicated (cheap).

v2 design (vs the v1 baseline at 228 us):
  - unnormalized-exponential attention: logits are computed twice on the
    PE straight from a DMA-transposed morphosyn tile (no PE transposes):
      b-layout  [b,v]: lhsT=morT chunk, rhs=W    -> exp accum_out = Z_b
      vT-layout [v,b]: lhsT=W chunk,  rhs=morT   -> exp -> fp8 E tiles
    The softmax 1/Z_b is folded into the per-partition scale of the
    ScalarE PSUM drain, so attn is never normalized explicitly.
  - fp8 DoubleRow matmuls (contraction 256/instruction) on E x vocab_fp8,
    weight-stationary inner order (rhs-column loop inside each weight).
  - ScalarE drains PSUM -> bf16 raw (scale=1/Z); DVE does
    sub (raw-stem), then wC-gated mul+add, all bf16 SBUF at 2x.
  - pivot softmax uses exp-with-accum (no DVE reduce), STT chain, scan.
  - collective doorbell as early as possible; the main loop's sub work
    is wC-independent and fills the AllGather latency.
  - vocab resident in SBUF as fp8 (4.2 MB), pivot/morphosyn/W bf16.
"""

import os
import sys

import numpy as np

for _p in ("/opt/trn_rl_repo",):
    if os.path.isdir(_p) and _p not in sys.path:
        sys.path.append(_p)

import concourse.bass as bass  # noqa: E402
import concourse.tile as tile  # noqa: E402
from concourse import bacc, mybir  # noqa: E402
from concourse.bass import ts  # noqa: E402
from concourse.bass_utils import run_bass_kernel_spmd  # noqa: E402
from concourse.masks import make_identity  # noqa: E402

import ml_dtypes  # noqa: E402

B, D, N, DM, V = 1024, 256, 256, 128, 512
NCORES = 8
DLOC = D // NCORES          # 32 d-values per core
BCH = B // 128              # 8 batch chunks
DN = DLOC * N               # 8192 free elems per core
HALF = DN // 2              # 4096 per round (16 d-values)
DHALF = DLOC // 2
PSW = 2048                  # one psum tile = 4 banks

F32 = mybir.dt.float32
BF16 = mybir.dt.bfloat16
FP8 = mybir.dt.float8e4
EXP = mybir.ActivationFunctionType.Exp
COPY = mybir.ActivationFunctionType.Copy
ALU = mybir.AluOpType
DR = mybir.MatmulPerfMode.DoubleRow

LAST_RESULT = None

_CACHE = {}


def _build():
    if "nc" in _CACHE:
        return _CACHE["nc"]

    nc = bacc.Bacc("TRN2", target_bir_lowering=False, debug=False,
                   num_devices=NCORES)

    stem_d = nc.dram_tensor("stem", [B, DLOC, N], BF16, kind="ExternalInput").ap()
    vocab_d = nc.dram_tensor("vocab", [V, DLOC, N], FP8, kind="ExternalInput").ap()
    mor_d = nc.dram_tensor("morpho", [B, DM], BF16, kind="ExternalInput").ap()
    waff_d = nc.dram_tensor("waffix", [DM, V], BF16, kind="ExternalInput").ap()
    pv_d = nc.dram_tensor("pivot", [2, 2, 128, 5, N], BF16, kind="ExternalInput").ap()
    abf_d = nc.dram_tensor("abf", [1, 9], F32, kind="ExternalInput").ap()
    out_d = nc.dram_tensor("out", [B, DLOC, N], BF16, kind="ExternalOutput").ap()

    from contextlib import ExitStack

    with tile.TileContext(nc) as tc, ExitStack() as ctx:
        const = ctx.enter_context(tc.tile_pool(name="const", bufs=1))

        morT = const.tile([128, B], BF16)          # morphosyn^T, resident
        wsb = const.tile([128, V], BF16)           # W_affix, resident
        attnT = const.tile([128, 4, B], FP8)       # E tiles [v_part, vc, b]
        wc_sb = const.tile([128, BCH, N], BF16)    # gathered wC [b_part, cb, n]
        w_bcast = const.tile([128, 20], F32)
        sEb = const.tile([128, BCH], F32)          # Z per (b_part, cb)
        nsE = const.tile([128, BCH], F32)          # -Z
        zb32 = const.tile([128, BCH], F32)         # f32(bf16(Z))
        rZ = const.tile([128, BCH], F32)           # 1/f32(bf16(Z))
        ident = const.tile([128, 128], F32)
        diag = const.tile([128, BCH, 128], BF16)   # -Z_b identity per chunk

        make_identity(nc, ident[:, :])

        # vocab resident as fp8: [v_part, vc, r, (d n)]
        vqp = ctx.enter_context(tc.tile_pool(name="vq", bufs=1))
        vq = vqp.tile([128, 4, 2, HALF], FP8)

        small = ctx.enter_context(tc.tile_pool(name="small", bufs=1))
        pvp = tc.alloc_tile_pool(name="pv", bufs=1)
        pv = pvp.tile([128, 4, 5, N], BF16)
        pvE = pvp.tile([128, 20, N], F32)

        # ---------- tiny DMAs first: pivot path is the critical path ----
        abf = small.tile([1, 9], F32)
        nc.sync.dma_start(abf[0:1, :], abf_d[:, :])
        for ij in range(4):
            i, j = divmod(ij, 2)
            nc.sync.dma_start(pv[:, ij, :, :], pv_d[i, j, :, :, :])
        nc.sync.dma_start_transpose(morT[:, :], mor_d[:, :])
        nc.sync.dma_start(wsb[:, :], waff_d[:, :])

        # ---------- phase A: mixture weights a (x) b (x) f -> w_bcast ----
        eabf = small.tile([1, 9], F32)
        sums = small.tile([1, 3], F32)
        nc.scalar.activation(eabf[0:1, 0:2], abf[0:1, 0:2], EXP, accum_out=sums[0:1, 0:1])
        nc.scalar.activation(eabf[0:1, 2:4], abf[0:1, 2:4], EXP, accum_out=sums[0:1, 1:2])
        nc.scalar.activation(eabf[0:1, 4:9], abf[0:1, 4:9], EXP, accum_out=sums[0:1, 2:3])
        rsum = small.tile([1, 3], F32)
        nc.vector.reciprocal(rsum[0:1, :], sums[0:1, :])
        t4 = small.tile([1, 4], F32)
        nc.vector.tensor_mul(
            t4[0:1, :].rearrange("p (i j) -> p i j", i=2),
            eabf[0:1, 0:2].rearrange("p (i j) -> p i j", j=1).to_broadcast((1, 2, 2)),
            eabf[0:1, 2:4].rearrange("p (i j) -> p i j", i=1).to_broadcast((1, 2, 2)),
        )
        t20 = small.tile([1, 20], F32)
        nc.vector.tensor_mul(
            t20[0:1, :].rearrange("p (g k) -> p g k", g=4),
            t4[0:1, :].rearrange("p (g k) -> p g k", k=1).to_broadcast((1, 4, 5)),
            eabf[0:1, 4:9].rearrange("p (g k) -> p g k", g=1).to_broadcast((1, 4, 5)),
        )
        rr = small.tile([1, 1], F32)
        nc.vector.tensor_mul(rr[0:1, :], rsum[0:1, 0:1], rsum[0:1, 1:2])
        rrr = small.tile([1, 1], F32)
        nc.vector.tensor_mul(rrr[0:1, :], rr[0:1, :], rsum[0:1, 2:3])
        w20 = small.tile([1, 20], F32)
        nc.vector.tensor_scalar_mul(w20[0:1, :], t20[0:1, :], rrr[0:1, 0:1])
        nc.gpsimd.partition_broadcast(w_bcast[:, :], w20[0:1, :])

        # ---------- phase C: pivot softmaxes -> wC (this core's chunk) ----
        # per-group reciprocal, pipelined against the exps (no global barrier)
        sP = pvp.tile([128, 20], F32)
        rPw = pvp.tile([128, 20], F32)
        accA = pvp.tile([128, N], F32)
        accB = pvp.tile([128, N], F32)
        cur, nxt = accA, accB
        for g in range(20):
            nc.scalar.activation(pvE[:, g, :], pv[:, g // 5, g % 5, :], EXP,
                                 accum_out=sP[:, g:g + 1])
            nc.vector.reciprocal(rPw[:, g:g + 1], sP[:, g:g + 1])
            nc.vector.tensor_mul(rPw[:, g:g + 1], rPw[:, g:g + 1],
                                 w_bcast[:, g:g + 1])
            if g == 0:
                nc.vector.tensor_scalar_mul(accA[:, :], pvE[:, 0, :], rPw[:, 0:1])
            else:
                nc.vector.scalar_tensor_tensor(
                    out=nxt[:, :], in0=pvE[:, g, :], scalar=rPw[:, g:g + 1],
                    in1=cur[:, :], op0=ALU.mult, op1=ALU.add,
                )
                cur, nxt = nxt, cur
        wCl = pvp.tile([128, N], BF16)
        nc.vector.tensor_tensor_scan(
            wCl[:, :], data0=cur[:, :], data1=cur[:, :], initial=0.0,
            op0=ALU.add, op1=ALU.bypass,
        )
        dram = ctx.enter_context(tc.tile_pool(name="dram", bufs=1, space="DRAM"))
        wc_in = dram.tile([128, N], BF16)
        wc_out = nc.dram_tensor("wc_gath", [B, N], BF16, addr_space="Shared").ap()
        nc.sync.dma_start(wc_in[:, :], wCl[:, :])
        nc.gpsimd.collective_compute(
            "AllGather", ALU.bypass,
            replica_groups=[list(range(NCORES))],
            ins=[wc_in[:, :].opt()], outs=[wc_out[:, :].opt()],
        )
        nc.sync.dma_start(
            wc_sb[:, :, :],
            wc_out[:, :].rearrange("(c p) n -> p c n", p=128),
        )

        # ---------- vocab + first stem loads (after the collective's DMA) --
        for vc in range(4):
            for r in range(2):
                nc.sync.dma_start(
                    vq[:, vc, r, :],
                    vocab_d[ts(vc, 128), ts(r, DHALF), :].rearrange("p d n -> p (d n)"),
                )

        # ---------- attention: logits both layouts, exp, Z ----------
        psA = tc.alloc_tile_pool(name="psA", bufs=2, space="PSUM")
        psB = tc.alloc_tile_pool(name="psB", bufs=2, space="PSUM")
        ebp = tc.alloc_tile_pool(name="eb", bufs=2)

        # vT-layout: E tiles for the big matmul (fp8, unnormalized)
        for vc in range(4):
            for bh in range(2):
                lgT = psA.tile([128, 512], F32, tag="lgT", name=f"lgT{vc}_{bh}")
                nc.tensor.matmul(lgT[:, :], lhsT=wsb[:, ts(vc, 128)],
                                 rhs=morT[:, ts(bh, 512)], start=True, stop=True)
                nc.scalar.activation(attnT[:, vc, ts(bh, 512)], lgT[:, :], EXP)
        # b-layout: only for Z_b = sum_v e^logit
        for cb in range(BCH):
            lgb = psB.tile([128, V], F32, tag="lgb", name=f"lgb{cb}")
            nc.tensor.matmul(lgb[:, :], lhsT=morT[:, ts(cb, 128)],
                             rhs=wsb[:, :], start=True, stop=True)
            eb = ebp.tile([128, V], BF16, tag="eb", name=f"eb{cb}")
            nc.scalar.activation(eb[:, :], lgb[:, :], EXP,
                                 accum_out=sEb[:, cb:cb + 1])
        # rZ inverts the bf16-rounded Z the diag tiles carry, so the
        # -Z*stem matmul term cancels exactly in delta = ps*rZ
        zb16 = ebp.tile([128, BCH], BF16, tag="zb16", name="zb16")
        nc.vector.tensor_copy(zb16[:, :], sEb[:, :])
        nc.vector.tensor_copy(zb32[:, :], zb16[:, :])
        nc.vector.reciprocal(rZ[:, :], zb32[:, :])
        nc.vector.tensor_scalar_mul(nsE[:, :], sEb[:, :], -1.0)
        for cb in range(BCH):
            nc.vector.tensor_scalar_mul(diag[:, cb, :], ident[:, :],
                                        nsE[:, cb:cb + 1])

        ebp.release()
        pvp.release()
        psB.release()
        psA.release()

        # ---------- main loop ----------
        stp = ctx.enter_context(tc.tile_pool(name="stem", bufs=6))
        rwp = ctx.enter_context(tc.tile_pool(name="raw", bufs=2))
        dlp = ctx.enter_context(tc.tile_pool(name="delta", bufs=6))
        prp = ctx.enter_context(tc.tile_pool(name="prod", bufs=2))
        otp = ctx.enter_context(tc.tile_pool(name="outp", bufs=2))
        psD = ctx.enter_context(tc.tile_pool(name="psD", bufs=2, space="PSUM"))

        for cb in range(BCH):
            for r in range(2):
                stem_t = stp.tile([128, HALF], BF16)
                nc.sync.dma_start(
                    stem_t[:, :],
                    stem_d[ts(cb, 128), ts(r, DHALF), :].rearrange("p d n -> p (d n)"),
                )
                # late rounds fold -Z_b*stem into PSUM on the (otherwise
                # idle) PE, so their post-wC DVE path skips the subtract
                on_pe = cb >= 4
                delta_t = dlp.tile([128, HALF], BF16)
                if not on_pe:
                    raw_t = rwp.tile([128, HALF], BF16)
                for h in range(2):
                    ps = psD.tile([128, PSW], F32)
                    # weight-stationary: each DoubleRow pair (contraction 256)
                    # sweeps all 4 psum banks before switching weights
                    for g in range(2):
                        for t in range(PSW // 512):
                            col = h * PSW + t * 512
                            nc.tensor.matmul(
                                ps[:, ts(t, 512)],
                                lhsT=attnT[:, 2 * g:2 * g + 2, ts(cb, 128)],
                                rhs=vq[:, 2 * g:2 * g + 2, r, col:col + 512],
                                start=(g == 0),
                                stop=(g == 1 and not on_pe),
                                perf_mode=DR,
                            )
                    if on_pe:
                        for t in range(PSW // 512):
                            nc.tensor.matmul(
                                ps[:, ts(t, 512)],
                                lhsT=diag[:, cb, :],
                                rhs=stem_t[:, h * PSW + t * 512:h * PSW + (t + 1) * 512],
                                start=False, stop=(t == 3),
                            )
                        nc.scalar.activation(delta_t[:, ts(h, PSW)], ps[:, :],
                                             COPY, scale=rZ[:, cb:cb + 1])
                    else:
                        nc.scalar.activation(raw_t[:, ts(h, PSW)], ps[:, :],
                                             COPY, scale=rZ[:, cb:cb + 1])
                if not on_pe:
                    # full-round [128,4096] subtract (wC-independent)
                    nc.vector.tensor_sub(delta_t[:, :], raw_t[:, :], stem_t[:, :])
                prod = prp.tile([128, HALF], BF16)
                nc.vector.tensor_mul(
                    prod[:, :].rearrange("p (a n) -> p a n", n=N),
                    delta_t[:, :].rearrange("p (a n) -> p a n", n=N),
                    wc_sb[:, cb:cb + 1, :].to_broadcast((128, HALF // N, N)),
                )
                out_t = otp.tile([128, HALF], BF16)
                nc.vector.tensor_add(out_t[:, :], prod[:, :], stem_t[:, :])
                nc.sync.dma_start(
                    out_d[ts(cb, 128), bass.ds(r * DHALF, DHALF), :]
                    .rearrange("p d n -> p (d n)"),
                    out_t[:, :],
                )

    nc.compile()
    _CACHE["nc"] = nc
    return nc


def kernel(stem_form, morphosyn, pivot_logits, W_affix, affix_vocab,
           alpha, beta, phi, max_len):
    global LAST_RESULT
    stem_form = np.asarray(stem_form, dtype=np.float32)
    morphosyn = np.asarray(morphosyn, dtype=np.float32)
    pivot_logits = np.asarray(pivot_logits, dtype=np.float32)
    W_affix = np.asarray(W_affix, dtype=np.float32)
    affix_vocab = np.asarray(affix_vocab, dtype=np.float32)
    abf = np.concatenate([
        np.asarray(alpha, np.float32).ravel(),
        np.asarray(beta, np.float32).ravel(),
        np.asarray(phi, np.float32).ravel(),
    ]).reshape(1, 9)

    nc = _build()

    stem_np = stem_form.astype(ml_dtypes.bfloat16)
    vocab_np = affix_vocab.astype(ml_dtypes.float8_e4m3)
    mor_np = morphosyn.astype(ml_dtypes.bfloat16)
    waff_np = W_affix.astype(ml_dtypes.bfloat16)
    pv_np = pivot_logits.astype(ml_dtypes.bfloat16)

    in_maps = []
    for c in range(NCORES):
        dlo, dhi = c * DLOC, (c + 1) * DLOC
        in_maps.append({
            "stem": np.ascontiguousarray(stem_np[:, dlo:dhi, :]),
            "vocab": np.ascontiguousarray(vocab_np[:, dlo:dhi, :]),
            "morpho": mor_np,
            "waffix": waff_np,
            "pivot": np.ascontiguousarray(pv_np[:, :, c * 128:(c + 1) * 128, :, :]),
            "abf": abf,
        })

    LAST_RESULT = run_bass_kernel_spmd(nc, in_maps, core_ids=list(range(NCORES)))
    outs = [LAST_RESULT.results[c]["out"] for c in range(NCORES)]
    out = np.concatenate([o.astype(np.float32) for o in outs], axis=1)
    return np.ascontiguousarray(out)
